# revision 1
# baseline (speedup 1.0000x reference)
"""Distributed ARMAConv kernel for 8 TRN2 NeuronCores (Bass/Tile).

Reference computation (N=16384 nodes, F=64 in-feats, C=32 channels,
K=2 stacks, T=2 iterations):
    for each stack k:  xbar = x
        for i in 0..1: xbar = relu(fltr @ (xbar @ w1) + x @ w2 + b)
    out = mean over stacks                                  -> [N, 32]

Strategy (measured 884 us on HW vs ~800 us per-core HBM roofline):
  - Row-shard fltr across 8 cores; core m holds fltr[rows_m, :] stored
    TRANSPOSED (contraction-major, split into two contiguous half-arrays)
    so every TensorE tile is a large contiguous DMA read.
  - Fuse the two independent ARMA stacks: Y = [xbar_k0 @ w1_k0 |
    xbar_k1 @ w1_k1] is [N, 64], so fltr streams from HBM only twice
    (once per iteration) instead of four times - the memory roofline.
  - Iteration 0 needs no communication (x is replicated).  Between the
    iterations, Y1 = xbar1 @ w1 ([N, 64]) is all-gathered.  Pass 1 runs
    in two output-row halves so the first half's gather fires at
    mid-stream and hides completely; collective_compute blocks the
    gpsimd queue (which also issues the cast-DMAs), so the next phase's
    fltr tiles are prefetched ahead of each collective in queue order.
  - fltr is read from HBM at full f32 width but cast to bf16 inside the
    DMA datapath (gpsimd SWDGE cast-DMA) so the TensorEngine runs at
    1 cyc/row; fp32r keeps full precision for the small skip-term
    matmuls.  bf16 conv + f32 PSUM accumulate gives rel err ~2e-3.
  - All big matmuls run transposed (out^T = Y^T @ fltr_m^T) so the
    moving operand streams 512 rows/instr; Y tiles are the stationary
    operand (weight loads hide under the previous matmul).
  - Pass 2 runs in two output-column halves so the first epilogue hides
    under the second stream; pass 1's last four tiles stay pinned in
    SBUF and pass 2 consumes them first in its second half (8 MiB of
    HBM reads saved and no DMA tail).
  - relu positive homogeneity folds the final stack-mean 0.5 scale into
    the pass-2 activation; the host only shards/transposes inputs and
    concatenates/transposes the [32, 2048] per-core outputs.
"""

import numpy as np

import concourse.mybir as mybir
import concourse.tile as tile
from concourse import bacc
from concourse.bass_utils import run_bass_kernel_spmd

N = 16384            # nodes
F = 64               # input features
C = 32               # channels per stack
C2 = 2 * C           # fused channels (2 stacks)
NCORES = 8
R = N // NCORES      # fltr rows per core (2048)
P = 128              # partitions
NKT = N // P         # K tiles per full pass (128)
RC = 4               # output row chunks per core
RCW = R // RC        # 512
XCHUNK = 1024        # xT DMA chunk width
KB1 = 4              # K tiles per pass-1 fltr DMA (4 MiB f32 reads)

F32 = mybir.dt.float32
F32R = mybir.dt.float32r
BF16 = mybir.dt.bfloat16

_CACHE = {}


def _build():
    nc = bacc.Bacc(
        trn_type="TRN2", target_bir_lowering=False, debug=False,
        num_devices=NCORES,
    )
    fltrT0_e = nc.dram_tensor("fltrt0", [N, R // 2], F32, kind="ExternalInput")
    fltrT1_e = nc.dram_tensor("fltrt1", [N, R // 2], F32, kind="ExternalInput")
    xT_e = nc.dram_tensor("xt", [F, N], F32, kind="ExternalInput")
    xtm_e = nc.dram_tensor("xtm", [F, R], F32, kind="ExternalInput")
    w1i0_e = nc.dram_tensor("w1i0", [F, C2], F32, kind="ExternalInput")
    w1i1_e = nc.dram_tensor("w1i1", [C2, C2], F32, kind="ExternalInput")
    w2i0_e = nc.dram_tensor("w2i0", [F, C2], F32, kind="ExternalInput")
    w2i1_e = nc.dram_tensor("w2i1", [F, C2], F32, kind="ExternalInput")
    bi0_e = nc.dram_tensor("bi0", [C2, 1], F32, kind="ExternalInput")
    bi1h_e = nc.dram_tensor("bi1h", [C2, 1], F32, kind="ExternalInput")
    out_e = nc.dram_tensor("out", [C, R], F32, kind="ExternalOutput")

    RG = [list(range(NCORES))]

    with tile.TileContext(nc) as tc:
        with (
            tc.tile_pool(name="wpool", bufs=1) as wpool,
            tc.tile_pool(name="xcpool", bufs=2) as xcpool,
            tc.tile_pool(name="y0pool", bufs=1) as y0pool,
            tc.tile_pool(name="fpool", bufs=4) as fpool,
            tc.tile_pool(name="xbpool", bufs=2) as xbpool,
            tc.tile_pool(name="ylpool", bufs=2) as ylpool,
            tc.tile_pool(name="ygpool", bufs=3) as ygpool,
            tc.tile_pool(name="opool", bufs=1) as opool,
            tc.tile_pool(name="pacc", bufs=4, space="PSUM") as pacc,
            tc.tile_pool(name="psmall", bufs=2, space="PSUM") as psmall,
            tc.tile_pool(name="dram", bufs=8, space="DRAM") as dram,
        ):
            # w1i0 and the first xT chunk first: they gate the first
            # Y0 matmul and thus the whole pass-1 PE start
            w1i0 = wpool.tile([F, C2], F32)
            nc.sync.dma_start(w1i0[:], w1i0_e[:])
            xc0 = xcpool.tile([F, XCHUNK], F32, name="xc")
            nc.sync.dma_start(xc0[:], xT_e[:, 0:XCHUNK])

            # remaining resident small tensors
            w1i1 = wpool.tile([C2, C2], F32)  # block-diag [w1_k0i1, w1_k1i1]
            nc.sync.dma_start(w1i1[:], w1i1_e[:])
            w2i0 = wpool.tile([F, C2], F32R)
            nc.sync.dma_start(w2i0[:], w2i0_e[:].bitcast(F32R))
            w2i1 = wpool.tile([F, C2], F32R)
            nc.sync.dma_start(w2i1[:], w2i1_e[:].bitcast(F32R))
            bi0 = wpool.tile([C2, 1], F32)
            nc.sync.dma_start(bi0[:], bi0_e[:])
            bi1h = wpool.tile([C2, 1], F32)
            nc.sync.dma_start(bi1h[:], bi1h_e[:])
            xm = wpool.tile([F, R], F32R)
            nc.sync.dma_start(xm[:], xtm_e[:].bitcast(F32R))

            y0 = y0pool.tile([P, NKT, C2], BF16, tag="y0")  # node-major Y0

            # ---- Y0 = x @ [w1_k0i0 | w1_k1i0], node-major, cast to bf16 ----
            for g in range(N // XCHUNK):  # 16 groups of 8 kt
                if g == 0:
                    xc = xc0
                else:
                    xc = xcpool.tile([F, XCHUNK], F32, name="xc")
                    nc.sync.dma_start(xc[:],
                                      xT_e[:, g * XCHUNK:(g + 1) * XCHUNK])
                ps0 = psmall.tile([P, 8, C2], F32, name="ps0", tag="ps0")
                for i in range(8):
                    nc.tensor.matmul(
                        ps0[:, i, :],
                        xc[:, i * P:(i + 1) * P],
                        w1i0[:],
                        start=True, stop=True,
                    )
                nc.vector.tensor_copy(y0[:, g * 8:(g + 1) * 8, :], ps0[:])

            # ---- pass 1 in two row-halves: each half's single all-gather
            # ---- fires at mid-stream; the next phase's fltr DMAs are
            # ---- prefetched ahead of the collective on the gpsimd queue
            HW_ = R // 2          # 1024 output rows per half
            NB2G = HW_ // P       # 8 K-tiles per (half, core) gather block
            NKB1 = NKT // KB1     # 32 fltr DMAs per half
            PF = 6                # half-1 tiles prefetched before gather 0
            gouts = []
            pf_tiles = []

            def p1_conv(p1, ft, ktb):
                for b in range(KB1):
                    kt = ktb * KB1 + b
                    for rc2 in range(2):
                        nc.tensor.matmul(
                            p1[rc2][:],
                            y0[:, kt, :],
                            ft[:, b, rc2 * RCW:(rc2 + 1) * RCW],
                            start=False, stop=(kt == NKT - 1),
                        )

            fltr_halves = [fltrT0_e, fltrT1_e]

            def ft_dma(half, ktb):
                ft = fpool.tile([P, KB1, HW_], BF16, name="ft", tag="ft",
                                bufs=4)
                nc.gpsimd.dma_start(
                    ft[:],
                    fltr_halves[half][ktb * KB1 * P:(ktb + 1) * KB1 * P, :]
                    .rearrange("(b p) c -> p b c", p=P),
                )
                return ft

            def ft2_dma(h, j, q, oh):
                ft = fpool.tile([P, 4, HW_], BF16, name="ft2",
                                tag="ft2", bufs=5)
                base = j * R + h * HW_ + q * (HW_ // 2)
                nc.gpsimd.dma_start(
                    ft[:],
                    fltr_halves[oh][base:base + HW_ // 2, :]
                    .rearrange("(b p) c -> p b c", p=P),
                )
                return ft

            for half in range(2):
                p1 = []
                for rc2 in range(2):
                    rc = half * 2 + rc2
                    acc = pacc.tile([C2, RCW], F32, name=f"p1_{rc}", tag="acc")
                    nc.tensor.matmul(
                        acc[:],
                        w2i0[:],
                        xm[:, rc * RCW:(rc + 1) * RCW],
                        start=True, stop=False,
                    )
                    p1.append(acc)

                kept = {}
                for ktb in range(NKB1):
                    if half == 1 and ktb < PF:
                        ft = pf_tiles[ktb]
                    elif half == 1 and ktb >= NKB1 - 8:
                        # pin the tiles pass 2 needs for its (oh=1, j in
                        # {6, 7}) groups so they are not re-read from HBM
                        ft = fpool.tile([P, KB1, HW_], BF16, name="ftk",
                                        tag="ftk", bufs=8)
                        nc.gpsimd.dma_start(
                            ft[:],
                            fltrT1_e[ktb * KB1 * P:(ktb + 1) * KB1 * P, :]
                            .rearrange("(b p) c -> p b c", p=P),
                        )
                        kept[ktb] = ft
                    else:
                        ft = ft_dma(half, ktb)
                    p1_conv(p1, ft, ktb)
                if half == 1:
                    kept_tiles = kept

                if half == 0:
                    # prefetch half-1's first tiles so the SDMA engines stay
                    # fed while the collective blocks the gpsimd queue
                    pf_tiles = [ft_dma(1, k) for k in range(PF)]
                else:
                    # prefetch pass-2's first tiles for the same reason
                    pf2_tiles = [ft2_dma(0, 0, 0, 0), ft2_dma(0, 0, 1, 0),
                                 ft2_dma(0, 1, 0, 0), ft2_dma(0, 1, 1, 0)]

                # epilogue: relu -> Y1 local (bf16) -> one all-gather
                y1h = ylpool.tile([P, 8, C2], BF16, name="y1h")
                for rc2 in range(2):
                    rc = half * 2 + rc2
                    xb1 = xbpool.tile([C2, RCW], F32, name="xb1")
                    nc.scalar.activation(
                        xb1[:], p1[rc2][:], mybir.ActivationFunctionType.Relu,
                        bias=bi0[:], scale=1.0,
                    )
                    for t in range(RC):  # node-subtiles of 128 in the chunk
                        psy = psmall.tile([P, C2], F32, name="psy", tag="psy")
                        nc.tensor.matmul(
                            psy[:],
                            xb1[:, t * P:(t + 1) * P],
                            w1i1[:],
                            start=True, stop=True,
                        )
                        nc.vector.tensor_copy(y1h[:, rc2 * RC + t, :], psy[:])
                gin = dram.tile([HW_, C2], BF16, name="gin", tag="gin", bufs=2)
                nc.sync.dma_start(
                    gin[:].rearrange("(t p) ch -> p t ch", p=P),
                    y1h[:],
                )
                gout = dram.tile(
                    [NCORES * HW_, C2], BF16, name="gout", tag="gout",
                    addr_space="Shared", bufs=2,
                )
                nc.gpsimd.collective_compute(
                    "AllGather", mybir.AluOpType.bypass,
                    replica_groups=RG,
                    ins=[gin[:].opt()], outs=[gout[:].opt()],
                )
                gouts.append(gout)

            outT = opool.tile([C, R], F32)

            # ---- pass 2: two output-column halves; the first half's
            # ---- epilogue hides under the second half's stream ----
            yg_all = y0pool.tile([P, NKT, C2], BF16, name="yg_all",
                                 tag="y0")
            nc.sync.dma_start(
                yg_all[:, 0:NKT // 2, :],
                gouts[0][:].rearrange("(b p) ch -> p b ch", p=P),
            )
            yg_h1_issued = [False]

            def issue_yg_h1():
                # deferred so pass-2's first matmuls don't transitively wait
                # on gather 1 through whole-tile dependency tracking
                if not yg_h1_issued[0]:
                    nc.sync.dma_start(
                        yg_all[:, NKT // 2:NKT, :],
                        gouts[1][:].rearrange("(b p) ch -> p b ch", p=P),
                    )
                    yg_h1_issued[0] = True

            for oh in range(2):
                p2 = []
                for rc2 in range(2):
                    rc = oh * 2 + rc2
                    acc = pacc.tile([C2, RCW], F32, name=f"p2_{rc}", tag="acc")
                    nc.tensor.matmul(
                        acc[:],
                        w2i1[:],
                        xm[:, rc * RCW:(rc + 1) * RCW],
                        start=True, stop=False,
                    )
                    p2.append(acc)
                jorder = list(range(NCORES)) if oh == 0 \
                    else [7, 0, 1, 2, 3, 4, 5, 6]
                n_done = 0
                for h in range(2):
                    if h == 1:
                        issue_yg_h1()
                    for j in jorder:
                        for q in range(2):
                            n_done += 1
                            if oh == 0 and h == 0 and j < 2:
                                ft = pf2_tiles[j * 2 + q]
                            elif oh == 1 and j >= 6:
                                # pinned from pass 1: ktb = j*4 + h*2 + q,
                                # cols [1024:2048]
                                ft = kept_tiles[j * 4 + h * 2 + q]
                            else:
                                ft = ft2_dma(h, j, q, oh)
                            for t in range(4):
                                kt_in = q * 4 + t
                                last = (n_done == 2 * NCORES * 2) and (t == 3)
                                for rc2 in range(2):
                                    nc.tensor.matmul(
                                        p2[rc2][:],
                                        yg_all[:, h * (NKT // 2)
                                               + j * NB2G + kt_in, :],
                                        ft[:, t, rc2 * RCW:(rc2 + 1) * RCW],
                                        start=False, stop=last,
                                    )
                # epilogue for this output half
                for rc2 in range(2):
                    rc = oh * 2 + rc2
                    xb2 = xbpool.tile([C2, RCW], F32, name="xb2")
                    nc.scalar.activation(
                        xb2[:], p2[rc2][:], mybir.ActivationFunctionType.Relu,
                        bias=bi1h[:], scale=0.5,
                    )
                    # partition-shift stack-1 half to base 0 (DMA), then add
                    xs = xbpool.tile([C, RCW], F32, name="xs")
                    nc.sync.dma_start(xs[:], xb2[C:C2, :])
                    nc.vector.tensor_add(
                        outT[:, rc * RCW:(rc + 1) * RCW],
                        xb2[0:C, :], xs[:],
                    )
                nc.sync.dma_start(
                    out_e[:, oh * HW_:(oh + 1) * HW_],
                    outT[:, oh * HW_:(oh + 1) * HW_],
                )

    nc.compile()
    return nc


def kernel(**inputs):
    x = np.ascontiguousarray(np.asarray(inputs["x"], dtype=np.float32))
    fltr = np.ascontiguousarray(np.asarray(inputs["fltr"], dtype=np.float32))

    def cat(a, b, axis=1):
        return np.ascontiguousarray(
            np.concatenate(
                [np.asarray(a, np.float32), np.asarray(b, np.float32)],
                axis=axis,
            )
        )

    w1i0 = cat(inputs["k0i0_w1"], inputs["k1i0_w1"])
    w1i1 = np.zeros((C2, C2), dtype=np.float32)
    w1i1[0:C, 0:C] = np.asarray(inputs["k0i1_w1"], np.float32)
    w1i1[C:C2, C:C2] = np.asarray(inputs["k1i1_w1"], np.float32)
    w2i0 = cat(inputs["k0i0_w2"], inputs["k1i0_w2"])
    w2i1 = cat(inputs["k0i1_w2"], inputs["k1i1_w2"])
    bi0 = cat(inputs["k0i0_b"], inputs["k1i0_b"], axis=0)[:, None]
    bi1h = 0.5 * cat(inputs["k0i1_b"], inputs["k1i1_b"], axis=0)[:, None]
    bi1h = np.ascontiguousarray(bi1h)
    xT = np.ascontiguousarray(x.T)

    if "nc" not in _CACHE:
        _CACHE["nc"] = _build()
    nc = _CACHE["nc"]

    in_maps = []
    for m in range(NCORES):
        rows = slice(m * R, (m + 1) * R)
        in_maps.append({
            "fltrt0": np.ascontiguousarray(fltr[m * R:m * R + R // 2, :].T),
            "fltrt1": np.ascontiguousarray(fltr[m * R + R // 2:(m + 1) * R, :].T),
            "xt": xT,
            "xtm": np.ascontiguousarray(x[rows, :].T),
            "w1i0": w1i0, "w1i1": w1i1, "w2i0": w2i0, "w2i1": w2i1,
            "bi0": bi0, "bi1h": bi1h,
        })

    import os
    import time
    trace = os.environ.get("ARMA_TRACE") == "1"
    last_exc = None
    for attempt in range(3):
        try:
            res = run_bass_kernel_spmd(
                nc, in_maps, core_ids=list(range(NCORES)), trace=trace,
            )
            break
        except Exception as e:  # transient NRT device errors: retry
            last_exc = e
            time.sleep(5.0)
    else:
        raise last_exc
    _CACHE["last_results"] = res
    out = np.concatenate(
        [np.asarray(res.results[m]["out"]).T for m in range(NCORES)], axis=0
    )
    return out



# revision 6
# speedup vs baseline: 2.3873x; 2.3873x over previous
"""Distributed ARMAConv kernel for 8 TRN2 NeuronCores (Bass/Tile).

Reference computation (N=16384 nodes, F=64 in-feats, C=32 channels,
K=2 stacks, T=2 iterations):
    for each stack k:  xbar = x
        for i in 0..1: xbar = relu(fltr @ (xbar @ w1) + x @ w2 + b)
    out = mean over stacks                                  -> [N, 32]

Strategy:
  - Row-shard fltr across 8 cores; core m holds fltr[rows_m, :] stored
    TRANSPOSED (contraction-major, two contiguous half-arrays) so every
    TensorE tile is a large contiguous DMA read.
  - fltr is stored at rest in DRAM as FP8 E3M4, pre-scaled by 2^8 on
    the host (the 2^-8 descale is folded into w1, exactly).  This cuts
    the dominant HBM stream 4x vs f32: 32 MiB per core per pass.  The
    PE consumes fp8 at bf16 speed (no DoubleRow - E4M3 would lose too
    much precision), so the kernel is TensorE-bound at ~110 us/pass.
  - Fuse the two independent ARMA stacks: Y = [xbar_k0 @ w1_k0 |
    xbar_k1 @ w1_k1] is [N, 64], so fltr streams only once per
    iteration.
  - All big matmuls run transposed (out^T = Y^T @ fltr_m^T) so fltr is
    the 1024-wide moving operand (128 elem/cycle); Y tiles are the
    stationary operand (weight loads hide under the previous matmul).
  - Iteration 0 needs no communication (x is replicated).  Pass 1 runs
    in two output-row halves so each half's Y1 all-gather fires at
    mid-stream and hides under the other half's matmuls.
  - Big fltr DMAs ride the sync-engine HWDGE ring; all small/latency
    DMAs ride the scalar-engine ring so they never queue behind a
    1 MiB fltr read; collectives keep the gpsimd queue.
  - relu positive homogeneity folds the final stack-mean 0.5 scale into
    the pass-2 activation; the host only shards/quantizes inputs and
    concatenates/transposes the [32, 2048] per-core outputs.
"""

import numpy as np
import ml_dtypes

import concourse.mybir as mybir
import concourse.tile as tile
from concourse import bacc
from concourse.bass_utils import run_bass_kernel_spmd

N = 16384            # nodes
F = 64               # input features
C = 32               # channels per stack
C2 = 2 * C           # fused channels (2 stacks)
NCORES = 8
R = N // NCORES      # fltr rows per core (2048)
P = 128              # partitions
NKT = N // P         # K tiles per full pass (128)
HW_ = R // 2         # 1024 output rows per half-array
XCHUNK = 1024        # xT DMA chunk width
KB = 8               # K tiles per fltr DMA (1 MiB fp8 reads)
FSCALE = 256.0       # power-of-2 fp8 pre-scale (folded into w1)

F32 = mybir.dt.float32
F32R = mybir.dt.float32r
BF16 = mybir.dt.bfloat16
F8 = mybir.dt.float8e3

_CACHE = {}


def _build():
    nc = bacc.Bacc(
        trn_type="TRN2", target_bir_lowering=False, debug=False,
        num_devices=NCORES,
    )
    fltrT0_e = nc.dram_tensor("fltrt0", [N, HW_], F8, kind="ExternalInput")
    fltrT1_e = nc.dram_tensor("fltrt1", [N, HW_], F8, kind="ExternalInput")
    xT_e = nc.dram_tensor("xt", [F, N], BF16, kind="ExternalInput")
    xtm_e = nc.dram_tensor("xtm", [F, R], F32, kind="ExternalInput")
    w1i0_e = nc.dram_tensor("w1i0", [F, C2], BF16, kind="ExternalInput")
    w1i1_e = nc.dram_tensor("w1i1", [C2, C2], BF16, kind="ExternalInput")
    w2i0_e = nc.dram_tensor("w2i0", [F, C2], F32, kind="ExternalInput")
    w2i1_e = nc.dram_tensor("w2i1", [F, C2], F32, kind="ExternalInput")
    bi0_e = nc.dram_tensor("bi0", [C2, 1], F32, kind="ExternalInput")
    bi1h_e = nc.dram_tensor("bi1h", [C2, 1], F32, kind="ExternalInput")
    out_e = nc.dram_tensor("out", [C, R], F32, kind="ExternalOutput")

    RG = [list(range(NCORES))]

    with tile.TileContext(nc) as tc:
        with (
            tc.tile_pool(name="wpool", bufs=1) as wpool,
            tc.tile_pool(name="xcpool", bufs=2) as xcpool,
            tc.tile_pool(name="y0pool", bufs=1) as y0pool,
            tc.tile_pool(name="ygpool", bufs=1) as ygpool,
            tc.tile_pool(name="fpool", bufs=4) as fpool,
            tc.tile_pool(name="xbpool", bufs=2) as xbpool,
            tc.tile_pool(name="ylpool", bufs=2) as ylpool,
            tc.tile_pool(name="opool", bufs=1) as opool,
            tc.tile_pool(name="pacc", bufs=4, space="PSUM") as pacc,
            tc.tile_pool(name="psmall", bufs=2, space="PSUM") as psmall,
            tc.tile_pool(name="dram", bufs=8, space="DRAM") as dram,
        ):
            # w1i0 and the first xT chunk first: they gate the first
            # Y0 matmul and thus the whole pass-1 PE start
            w1i0 = wpool.tile([F, C2], BF16)
            nc.scalar.dma_start(w1i0[:], w1i0_e[:])
            xc0 = xcpool.tile([F, XCHUNK], BF16, name="xc")
            nc.scalar.dma_start(xc0[:], xT_e[:, 0:XCHUNK])

            # remaining resident small tensors
            w1i1 = wpool.tile([C2, C2], BF16)  # block-diag [w1_k0i1, w1_k1i1]
            nc.scalar.dma_start(w1i1[:], w1i1_e[:])
            w2i0 = wpool.tile([F, C2], F32R)
            nc.scalar.dma_start(w2i0[:], w2i0_e[:].bitcast(F32R))
            w2i1 = wpool.tile([F, C2], F32R)
            nc.scalar.dma_start(w2i1[:], w2i1_e[:].bitcast(F32R))
            bi0 = wpool.tile([C2, 1], F32)
            nc.scalar.dma_start(bi0[:], bi0_e[:])
            bi1h = wpool.tile([C2, 1], F32)
            nc.scalar.dma_start(bi1h[:], bi1h_e[:])
            xm = wpool.tile([F, R], F32R)
            nc.scalar.dma_start(xm[:], xtm_e[:].bitcast(F32R))

            y0 = y0pool.tile([P, NKT, C2], BF16, tag="y0")  # node-major Y0

            # ---- Y0 = x @ [w1_k0i0 | w1_k1i0], node-major, cast to bf16 ----
            for g in range(N // XCHUNK):  # 16 groups of 8 kt
                if g == 0:
                    xc = xc0
                else:
                    xc = xcpool.tile([F, XCHUNK], BF16, name="xc")
                    nc.scalar.dma_start(xc[:],
                                        xT_e[:, g * XCHUNK:(g + 1) * XCHUNK])
                ps0 = psmall.tile([P, 8, C2], F32, name="ps0", tag="ps0")
                for i in range(8):
                    nc.tensor.matmul(
                        ps0[:, i, :],
                        xc[:, i * P:(i + 1) * P],
                        w1i0[:],
                        start=True, stop=True,
                    )
                nc.vector.tensor_copy(y0[:, g * 8:(g + 1) * 8, :], ps0[:])

            fltr_halves = [fltrT0_e, fltrT1_e]

            def ft_dma(half, row_lo):
                # one [128, KB, 1024] fp8 tile = KB k-tiles, 1 MiB
                ft = fpool.tile([P, KB, HW_], F8, name="ft", tag="ft")
                nc.sync.dma_start(
                    ft[:],
                    fltr_halves[half][row_lo:row_lo + KB * P, :]
                    .rearrange("(b p) c -> p b c", p=P),
                )
                return ft

            # ---- pass 1 in two output-row halves: each half's single
            # ---- all-gather fires at mid-stream and hides under the
            # ---- other half's matmul stream
            RCW = HW_ // 2  # 512 output rows per PSUM accumulator
            gouts = []
            for half in range(2):
                p1 = []
                for rc2 in range(2):
                    rc = half * 2 + rc2
                    acc = pacc.tile([C2, RCW], F32, name=f"p1_{rc}",
                                    tag="acc")
                    nc.tensor.matmul(
                        acc[:],
                        w2i0[:],
                        xm[:, rc * RCW:(rc + 1) * RCW],
                        start=True, stop=False,
                    )
                    p1.append(acc)
                for ktb in range(NKT // KB):
                    ft = ft_dma(half, ktb * KB * P)
                    for b in range(KB):
                        kt = ktb * KB + b
                        for rc2 in range(2):
                            nc.tensor.matmul(
                                p1[rc2][:],
                                y0[:, kt, :],
                                ft[:, b, rc2 * RCW:(rc2 + 1) * RCW],
                                start=False, stop=(kt == NKT - 1),
                            )

                # epilogue: relu -> Y1 local (bf16) -> one all-gather
                y1h = ylpool.tile([P, 8, C2], BF16, name="y1h")
                for rc2 in range(2):
                    xb1 = xbpool.tile([C2, RCW], BF16, name="xb1")
                    nc.scalar.activation(
                        xb1[:], p1[rc2][:], mybir.ActivationFunctionType.Relu,
                        bias=bi0[:], scale=1.0,
                    )
                    for t in range(4):  # node-subtiles of 128 in the chunk
                        psy = psmall.tile([P, C2], F32, name="psy", tag="psy")
                        nc.tensor.matmul(
                            psy[:],
                            xb1[:, t * P:(t + 1) * P],
                            w1i1[:],
                            start=True, stop=True,
                        )
                        nc.vector.tensor_copy(y1h[:, rc2 * 4 + t, :], psy[:])
                gin = dram.tile([HW_, C2], BF16, name="gin", tag="gin", bufs=2)
                nc.scalar.dma_start(
                    gin[:].rearrange("(t p) ch -> p t ch", p=P),
                    y1h[:],
                )
                gout = dram.tile(
                    [NCORES * HW_, C2], BF16, name="gout", tag="gout",
                    addr_space="Shared", bufs=2,
                )
                nc.gpsimd.collective_compute(
                    "AllGather", mybir.AluOpType.bypass,
                    replica_groups=RG,
                    ins=[gin[:].opt()], outs=[gout[:].opt()],
                )
                gouts.append(gout)

            outT = opool.tile([C, R], F32)

            # ---- pass 2: contraction is gathered Y1; separate yg tiles per
            # ---- gather half so oh=0's matmuls never wait on gather 1
            yg = [ygpool.tile([P, NKT // 2, C2], BF16, name=f"yg{h}",
                              tag=f"yg{h}") for h in range(2)]
            nc.scalar.dma_start(
                yg[0][:],
                gouts[0][:].rearrange("(b p) ch -> p b ch", p=P),
            )
            yg_h1_issued = [False]

            def issue_yg_h1():
                # deferred so pass-2's first matmuls are issued first and
                # the gather-1 wait overlaps them
                if not yg_h1_issued[0]:
                    nc.scalar.dma_start(
                        yg[1][:],
                        gouts[1][:].rearrange("(b p) ch -> p b ch", p=P),
                    )
                    yg_h1_issued[0] = True

            for oh in range(2):
                p2 = []
                for rc2 in range(2):
                    rc = oh * 2 + rc2
                    acc = pacc.tile([C2, RCW], F32, name=f"p2_{rc}",
                                    tag="acc")
                    nc.tensor.matmul(
                        acc[:],
                        w2i1[:],
                        xm[:, rc * RCW:(rc + 1) * RCW],
                        start=True, stop=False,
                    )
                    p2.append(acc)
                for h in range(2):
                    if h == 1:
                        issue_yg_h1()
                    for j in range(NCORES):
                        # gather block j covers fltrT rows
                        # [j*2048 + h*1024, +1024) = 8 k-tiles
                        ft = ft_dma(oh, j * R + h * HW_)
                        for b in range(KB):
                            last = (h == 1 and j == NCORES - 1
                                    and b == KB - 1)
                            for rc2 in range(2):
                                nc.tensor.matmul(
                                    p2[rc2][:],
                                    yg[h][:, j * KB + b, :],
                                    ft[:, b, rc2 * RCW:(rc2 + 1) * RCW],
                                    start=False, stop=last,
                                )
                # epilogue for this output half: relu(0.5*(z + b)) then
                # fold the two stacks: out = relu_k0 + relu_k1 (0.5 folded)
                for rc2 in range(2):
                    rc = oh * 2 + rc2
                    xb2 = xbpool.tile([C2, RCW], F32, name="xb2")
                    nc.scalar.activation(
                        xb2[:], p2[rc2][:], mybir.ActivationFunctionType.Relu,
                        bias=bi1h[:], scale=0.5,
                    )
                    # partition-shift stack-1 half to base 0 (DMA), then add
                    xs = xbpool.tile([C, RCW], F32, name="xs")
                    nc.scalar.dma_start(xs[:], xb2[C:C2, :])
                    nc.vector.tensor_add(
                        outT[:, rc * RCW:(rc + 1) * RCW],
                        xb2[0:C, :], xs[:],
                    )
                nc.scalar.dma_start(
                    out_e[:, oh * HW_:(oh + 1) * HW_],
                    outT[:, oh * HW_:(oh + 1) * HW_],
                )

    nc.compile()
    return nc


def kernel(**inputs):
    x = np.ascontiguousarray(np.asarray(inputs["x"], dtype=np.float32))
    fltr = np.ascontiguousarray(np.asarray(inputs["fltr"], dtype=np.float32))

    def cat(a, b, axis=1):
        return np.ascontiguousarray(
            np.concatenate(
                [np.asarray(a, np.float32), np.asarray(b, np.float32)],
                axis=axis,
            )
        )

    f8 = ml_dtypes.float8_e3m4
    bf = ml_dtypes.bfloat16
    w1i0 = np.ascontiguousarray(
        (cat(inputs["k0i0_w1"], inputs["k1i0_w1"]) / FSCALE).astype(bf))
    w1i1f = np.zeros((C2, C2), dtype=np.float32)
    w1i1f[0:C, 0:C] = np.asarray(inputs["k0i1_w1"], np.float32)
    w1i1f[C:C2, C:C2] = np.asarray(inputs["k1i1_w1"], np.float32)
    w1i1 = np.ascontiguousarray((w1i1f / FSCALE).astype(bf))
    w2i0 = cat(inputs["k0i0_w2"], inputs["k1i0_w2"])
    w2i1 = cat(inputs["k0i1_w2"], inputs["k1i1_w2"])
    bi0 = cat(inputs["k0i0_b"], inputs["k1i0_b"], axis=0)[:, None]
    bi1h = 0.5 * cat(inputs["k0i1_b"], inputs["k1i1_b"], axis=0)[:, None]
    bi1h = np.ascontiguousarray(bi1h)
    xT = np.ascontiguousarray(x.T.astype(bf))
    # fp8 E3M4 fltr at rest: transpose per core, scale by 2^8 (descale is
    # folded into w1i0/w1i1 above; values land in [-10.9, 10.9] < 15.5 max)
    fltrs = (fltr * np.float32(FSCALE)).astype(f8)

    if "nc" not in _CACHE:
        _CACHE["nc"] = _build()
    nc = _CACHE["nc"]

    in_maps = []
    for m in range(NCORES):
        rows = slice(m * R, (m + 1) * R)
        in_maps.append({
            "fltrt0": np.ascontiguousarray(fltrs[m * R:m * R + HW_, :].T),
            "fltrt1": np.ascontiguousarray(fltrs[m * R + HW_:(m + 1) * R, :].T),
            "xt": xT,
            "xtm": np.ascontiguousarray(x[rows, :].T),
            "w1i0": w1i0, "w1i1": w1i1, "w2i0": w2i0, "w2i1": w2i1,
            "bi0": bi0, "bi1h": bi1h,
        })

    import os
    import time
    trace = os.environ.get("ARMA_TRACE") == "1"
    last_exc = None
    for attempt in range(3):
        try:
            res = run_bass_kernel_spmd(
                nc, in_maps, core_ids=list(range(NCORES)), trace=trace,
            )
            break
        except Exception as e:  # transient NRT device errors: retry
            last_exc = e
            time.sleep(5.0)
    else:
        raise last_exc
    _CACHE["last_results"] = res
    out = np.concatenate(
        [np.asarray(res.results[m]["out"]).T for m in range(NCORES)], axis=0
    )
    return out


# revision 10
# speedup vs baseline: 2.4410x; 1.0225x over previous
"""Distributed ARMAConv kernel for 8 TRN2 NeuronCores (Bass/Tile).

Reference computation (N=16384 nodes, F=64 in-feats, C=32 channels,
K=2 stacks, T=2 iterations):
    for each stack k:  xbar = x
        for i in 0..1: xbar = relu(fltr @ (xbar @ w1) + x @ w2 + b)
    out = mean over stacks                                  -> [N, 32]

Strategy:
  - Row-shard fltr across 8 cores; core m holds fltr[rows_m, :] stored
    TRANSPOSED (contraction-major, two contiguous half-arrays) so every
    TensorE tile is a large contiguous DMA read.
  - fltr is stored at rest in DRAM as FP8 E3M4, pre-scaled by 2^8 on
    the host (the 2^-8 descale is folded into w1, exactly).  This cuts
    the dominant HBM stream 4x vs f32: 32 MiB per core per pass.  The
    PE consumes fp8 at bf16 speed (no DoubleRow - E4M3 would lose too
    much precision), so the kernel is TensorE-bound at ~110 us/pass.
  - Fuse the two independent ARMA stacks: Y = [xbar_k0 @ w1_k0 |
    xbar_k1 @ w1_k1] is [N, 64], so fltr streams only once per
    iteration.
  - All big matmuls run transposed (out^T = Y^T @ fltr_m^T) so fltr is
    the 512-wide moving operand (128 elem/cycle); Y tiles are the
    stationary operand (weight loads hide under the previous matmul).
  - Iteration 0 needs no communication (x is replicated).  Pass 1 runs
    as FOUR 512-row output chunks (column strips of the stored fltr^T),
    each followed by its own small (64 KiB) Y1 all-gather: the first
    fires ~60us in, the serial collective stream (each ~25us,
    latency-bound) hides completely under the remaining matmuls.
  - Pass 2 consumes the gathered chunks contraction-major (chunk 0..2
    feed all four output accumulators, chunk 3 is processed per output
    half so the first half's epilogue hides under the second half's
    stream); chunk 3 is not needed until ~85us after pass 2 starts,
    tolerating inter-core start skew.
  - Big fltr DMAs ride the sync-engine HWDGE ring; all small/latency
    DMAs ride the scalar-engine ring so they never queue behind a
    1 MiB fltr read; collectives keep the gpsimd queue.
  - relu positive homogeneity folds the final stack-mean 0.5 scale into
    the pass-2 activation; the host only shards/quantizes inputs and
    concatenates/transposes the [32, 2048] per-core outputs.
"""

import numpy as np
import ml_dtypes

import concourse.mybir as mybir
import concourse.tile as tile
from concourse import bacc
from concourse.bass_utils import run_bass_kernel_spmd

N = 16384            # nodes
F = 64               # input features
C = 32               # channels per stack
C2 = 2 * C           # fused channels (2 stacks)
NCORES = 8
R = N // NCORES      # fltr rows per core (2048)
P = 128              # partitions
NKT = N // P         # K tiles per full pass (128)
HW_ = R // 2         # 1024 output rows per half-array
CW = 512             # output rows per pass-1 chunk / PSUM accumulator
NCH = R // CW        # 4 pass-1 chunks (each with its own all-gather)
XCHUNK = 1024        # xT DMA chunk width
KB1 = 16             # K tiles per pass-1 fltr DMA (1 MiB fp8 reads)
FSCALE = 256.0       # power-of-2 fp8 pre-scale (folded into w1)

F32 = mybir.dt.float32
F32R = mybir.dt.float32r
BF16 = mybir.dt.bfloat16
F8 = mybir.dt.float8e3

_CACHE = {}


def _build():
    nc = bacc.Bacc(
        trn_type="TRN2", target_bir_lowering=False, debug=False,
        num_devices=NCORES,
    )
    fltrT0_e = nc.dram_tensor("fltrt0", [N, HW_], F8, kind="ExternalInput")
    fltrT1_e = nc.dram_tensor("fltrt1", [N, HW_], F8, kind="ExternalInput")
    xT_e = nc.dram_tensor("xt", [F, N], BF16, kind="ExternalInput")
    xtm_e = nc.dram_tensor("xtm", [F, R], F32, kind="ExternalInput")
    w1i0_e = nc.dram_tensor("w1i0", [F, C2], BF16, kind="ExternalInput")
    w1i1_e = nc.dram_tensor("w1i1", [C2, C2], BF16, kind="ExternalInput")
    w2i0_e = nc.dram_tensor("w2i0", [F, C2], F32, kind="ExternalInput")
    w2i1_e = nc.dram_tensor("w2i1", [F, C2], F32, kind="ExternalInput")
    bi0_e = nc.dram_tensor("bi0", [C2, 1], F32, kind="ExternalInput")
    bi1h_e = nc.dram_tensor("bi1h", [C2, 1], F32, kind="ExternalInput")
    out_e = nc.dram_tensor("out", [C, R], F32, kind="ExternalOutput")

    RG = [list(range(NCORES))]
    fltr_halves = [fltrT0_e, fltrT1_e]

    with tile.TileContext(nc) as tc:
        with (
            tc.tile_pool(name="wpool", bufs=1) as wpool,
            tc.tile_pool(name="xcpool", bufs=4) as xcpool,
            tc.tile_pool(name="y0pool", bufs=1) as y0pool,
            tc.tile_pool(name="ygpool", bufs=1) as ygpool,
            tc.tile_pool(name="fpool", bufs=6) as fpool,
            tc.tile_pool(name="xbpool", bufs=2) as xbpool,
            tc.tile_pool(name="ylpool", bufs=2) as ylpool,
            tc.tile_pool(name="opool", bufs=1) as opool,
            tc.tile_pool(name="pacc", bufs=4, space="PSUM") as pacc,
            tc.tile_pool(name="psmall", bufs=2, space="PSUM") as psmall,
            tc.tile_pool(name="dram", bufs=8, space="DRAM") as dram,
        ):
            # w1i0 and the first xT chunks first: they gate the first
            # Y0 matmul and thus the whole pass-1 PE start
            w1i0 = wpool.tile([F, C2], BF16)
            nc.scalar.dma_start(w1i0[:], w1i0_e[:])
            xcs = []
            for g in range(3):
                xc = xcpool.tile([F, XCHUNK], BF16, name="xc")
                nc.scalar.dma_start(xc[:],
                                    xT_e[:, g * XCHUNK:(g + 1) * XCHUNK])
                xcs.append(xc)

            # remaining resident small tensors
            w1i1 = wpool.tile([C2, C2], BF16)  # block-diag [w1_k0i1, w1_k1i1]
            nc.scalar.dma_start(w1i1[:], w1i1_e[:])
            w2i0 = wpool.tile([F, C2], F32R)
            nc.scalar.dma_start(w2i0[:], w2i0_e[:].bitcast(F32R))
            bi0 = wpool.tile([C2, 1], F32)
            nc.scalar.dma_start(bi0[:], bi0_e[:])
            xm = wpool.tile([F, R], F32R)
            nc.scalar.dma_start(xm[:], xtm_e[:].bitcast(F32R))
            w2i1 = wpool.tile([F, C2], F32R)
            nc.scalar.dma_start(w2i1[:], w2i1_e[:].bitcast(F32R))
            bi1h = wpool.tile([C2, 1], F32)
            nc.scalar.dma_start(bi1h[:], bi1h_e[:])

            y0 = y0pool.tile([P, NKT, C2], BF16, tag="y0")  # node-major Y0

            # ---- Y0 = x @ [w1_k0i0 | w1_k1i0], node-major, cast to bf16 ----
            for g in range(N // XCHUNK):  # 16 groups of 8 kt
                if g < 3:
                    xc = xcs[g]
                else:
                    xc = xcpool.tile([F, XCHUNK], BF16, name="xc")
                    nc.scalar.dma_start(xc[:],
                                        xT_e[:, g * XCHUNK:(g + 1) * XCHUNK])
                ps0 = psmall.tile([P, 8, C2], F32, name="ps0", tag="ps0")
                for i in range(8):
                    nc.tensor.matmul(
                        ps0[:, i, :],
                        xc[:, i * P:(i + 1) * P],
                        w1i0[:],
                        start=True, stop=True,
                    )
                nc.vector.tensor_copy(y0[:, g * 8:(g + 1) * 8, :], ps0[:])

            # pass-2 gather-chunk tiles: yg tile c holds gather chunk c; row
            # b*128+p of gout_c is node (b//4)*2048 + c*512 + (b%4)*128 + p.
            yg = [ygpool.tile([P, NCORES * 4, C2], BF16, name=f"yg{c}",
                              tag=f"yg{c}") for c in range(NCH)]
            yg_issued = [False] * NCH
            gouts = []

            def issue_yg(c):
                # deferred issue: by the time it is queued the gather is
                # (normally) complete, so the scalar ring never head-blocks
                if not yg_issued[c]:
                    nc.scalar.dma_start(
                        yg[c][:],
                        gouts[c][:].rearrange("(b p) ch -> p b ch", p=P),
                    )
                    yg_issued[c] = True

            def issue_yg0():
                issue_yg(0)

            # ---- pass 1: four 512-row output chunks, one small all-gather
            # ---- each; the serial collective stream hides under the matmuls
            for ck in range(NCH):
                hx, coff = ck // 2, (ck % 2) * CW
                acc = pacc.tile([C2, CW], F32, name=f"p1_{ck}", tag="acc")
                nc.tensor.matmul(
                    acc[:],
                    w2i0[:],
                    xm[:, ck * CW:(ck + 1) * CW],
                    start=True, stop=False,
                )
                for ktb in range(NKT // KB1):
                    ft = fpool.tile([P, KB1, CW], F8, name="ft", tag="ft")
                    nc.sync.dma_start(
                        ft[:],
                        fltr_halves[hx][ktb * KB1 * P:(ktb + 1) * KB1 * P,
                                        coff:coff + CW]
                        .rearrange("(b p) c -> p b c", p=P),
                    )
                    for b in range(KB1):
                        kt = ktb * KB1 + b
                        nc.tensor.matmul(
                            acc[:],
                            y0[:, kt, :],
                            ft[:, b, :],
                            start=False, stop=(kt == NKT - 1),
                        )

                if ck == NCH - 1:
                    # gather-0 finished long ago; load its yg tile now so
                    # pass-2's first matmuls start without a DMA bubble
                    issue_yg0()

                # epilogue: relu -> Y1 chunk (bf16) -> one small all-gather
                y1q = ylpool.tile([P, 4, C2], BF16, name="y1q")
                xb1 = xbpool.tile([C2, CW], BF16, name="xb1")
                nc.scalar.activation(
                    xb1[:], acc[:], mybir.ActivationFunctionType.Relu,
                    bias=bi0[:], scale=1.0,
                )
                for t in range(4):  # node-subtiles of 128 in the chunk
                    psy = psmall.tile([P, C2], F32, name="psy", tag="psy")
                    nc.tensor.matmul(
                        psy[:],
                        xb1[:, t * P:(t + 1) * P],
                        w1i1[:],
                        start=True, stop=True,
                    )
                    nc.vector.tensor_copy(y1q[:, t, :], psy[:])
                gin = dram.tile([CW, C2], BF16, name="gin", tag="gin", bufs=4)
                nc.scalar.dma_start(
                    gin[:].rearrange("(t p) ch -> p t ch", p=P),
                    y1q[:],
                )
                gout = dram.tile(
                    [NCORES * CW, C2], BF16, name="gout", tag="gout",
                    addr_space="Shared", bufs=4,
                )
                nc.gpsimd.collective_compute(
                    "AllGather", mybir.AluOpType.bypass,
                    replica_groups=RG,
                    ins=[gin[:].opt()], outs=[gout[:].opt()],
                )
                gouts.append(gout)

            outT = opool.tile([C, R], F32)

            # ---- pass 2: contraction is gathered Y1, consumed chunk-major
            p2 = []
            for rc in range(NCH):
                acc = pacc.tile([C2, CW], F32, name=f"p2_{rc}", tag="acc")
                nc.tensor.matmul(
                    acc[:],
                    w2i1[:],
                    xm[:, rc * CW:(rc + 1) * CW],
                    start=True, stop=False,
                )
                p2.append(acc)

            def p2_block(c, j, hx, rcs, stop):
                # contraction rows [j*2048 + c*512, +512) of half-array hx
                ft = fpool.tile([P, 4, HW_], F8, name="ft2", tag="ft")
                nc.sync.dma_start(
                    ft[:],
                    fltr_halves[hx][j * R + c * CW:j * R + (c + 1) * CW, :]
                    .rearrange("(b p) c -> p b c", p=P),
                )
                for b in range(4):
                    for rc in rcs:
                        nc.tensor.matmul(
                            p2[rc][:],
                            yg[c][:, j * 4 + b, :],
                            ft[:, b, (rc % 2) * CW:(rc % 2 + 1) * CW],
                            start=False,
                            stop=stop and b == 3 and rc == rcs[-1],
                        )

            # phase A: chunks 0..2 feed all four output accumulators
            for c in range(3):
                if c > 0:
                    issue_yg(c)
                for j in range(NCORES):
                    p2_block(c, j, 0, [0, 1], False)
                    p2_block(c, j, 1, [2, 3], False)

            def p2_epilogue(rc):
                xb2 = xbpool.tile([C2, CW], F32, name="xb2")
                nc.scalar.activation(
                    xb2[:], p2[rc][:], mybir.ActivationFunctionType.Relu,
                    bias=bi1h[:], scale=0.5,
                )
                # partition-shift stack-1 half to base 0 (DMA), then add
                xs = xbpool.tile([C, CW], F32, name="xs")
                nc.scalar.dma_start(xs[:], xb2[C:C2, :])
                nc.vector.tensor_add(
                    outT[:, rc * CW:(rc + 1) * CW],
                    xb2[0:C, :], xs[:],
                )

            # phase B: chunk 3 per output half; the first half's epilogue
            # hides under the second half's matmul stream
            issue_yg(3)
            for j in range(NCORES):
                p2_block(3, j, 0, [0, 1], j == NCORES - 1)
            for rc in (0, 1):
                p2_epilogue(rc)
            nc.scalar.dma_start(out_e[:, 0:HW_], outT[:, 0:HW_])
            for j in range(NCORES):
                p2_block(3, j, 1, [2, 3], j == NCORES - 1)
            for rc in (2, 3):
                p2_epilogue(rc)
            nc.scalar.dma_start(out_e[:, HW_:R], outT[:, HW_:R])

    nc.compile()
    return nc


def kernel(**inputs):
    x = np.ascontiguousarray(np.asarray(inputs["x"], dtype=np.float32))
    fltr = np.ascontiguousarray(np.asarray(inputs["fltr"], dtype=np.float32))

    def cat(a, b, axis=1):
        return np.ascontiguousarray(
            np.concatenate(
                [np.asarray(a, np.float32), np.asarray(b, np.float32)],
                axis=axis,
            )
        )

    f8 = ml_dtypes.float8_e3m4
    bf = ml_dtypes.bfloat16
    w1i0 = np.ascontiguousarray(
        (cat(inputs["k0i0_w1"], inputs["k1i0_w1"]) / FSCALE).astype(bf))
    w1i1f = np.zeros((C2, C2), dtype=np.float32)
    w1i1f[0:C, 0:C] = np.asarray(inputs["k0i1_w1"], np.float32)
    w1i1f[C:C2, C:C2] = np.asarray(inputs["k1i1_w1"], np.float32)
    w1i1 = np.ascontiguousarray((w1i1f / FSCALE).astype(bf))
    w2i0 = cat(inputs["k0i0_w2"], inputs["k1i0_w2"])
    w2i1 = cat(inputs["k0i1_w2"], inputs["k1i1_w2"])
    bi0 = cat(inputs["k0i0_b"], inputs["k1i0_b"], axis=0)[:, None]
    bi1h = 0.5 * cat(inputs["k0i1_b"], inputs["k1i1_b"], axis=0)[:, None]
    bi1h = np.ascontiguousarray(bi1h)
    xT = np.ascontiguousarray(x.T.astype(bf))
    # fp8 E3M4 fltr at rest: transpose per core, scale by 2^8 (descale is
    # folded into w1i0/w1i1 above; values land in [-10.9, 10.9] < 15.5 max)
    fltrs = (fltr * np.float32(FSCALE)).astype(f8)

    if "nc" not in _CACHE:
        _CACHE["nc"] = _build()
    nc = _CACHE["nc"]

    in_maps = []
    for m in range(NCORES):
        rows = slice(m * R, (m + 1) * R)
        in_maps.append({
            "fltrt0": np.ascontiguousarray(fltrs[m * R:m * R + HW_, :].T),
            "fltrt1": np.ascontiguousarray(fltrs[m * R + HW_:(m + 1) * R, :].T),
            "xt": xT,
            "xtm": np.ascontiguousarray(x[rows, :].T),
            "w1i0": w1i0, "w1i1": w1i1, "w2i0": w2i0, "w2i1": w2i1,
            "bi0": bi0, "bi1h": bi1h,
        })

    import os
    import time
    trace = os.environ.get("ARMA_TRACE") == "1"
    last_exc = None
    for attempt in range(3):
        try:
            res = run_bass_kernel_spmd(
                nc, in_maps, core_ids=list(range(NCORES)), trace=trace,
            )
            break
        except Exception as e:  # transient NRT device errors: retry
            last_exc = e
            time.sleep(5.0)
    else:
        raise last_exc
    _CACHE["last_results"] = res
    out = np.concatenate(
        [np.asarray(res.results[m]["out"]).T for m in range(NCORES)], axis=0
    )
    return out


# revision 18
# speedup vs baseline: 2.6441x; 1.0832x over previous
"""Distributed ARMAConv kernel for 8 TRN2 NeuronCores (Bass/Tile).

Reference computation (N=16384 nodes, F=64 in-feats, C=32 channels,
K=2 stacks, T=2 iterations):
    for each stack k:  xbar = x
        for i in 0..1: xbar = relu(fltr @ (xbar @ w1) + x @ w2 + b)
    out = mean over stacks                                  -> [N, 32]

Strategy:
  - Row-shard fltr across 8 cores; core m holds fltr[rows_m, :] stored
    TRANSPOSED (contraction-major, two contiguous half-arrays) so every
    TensorE tile is a large contiguous DMA read.
  - fltr is stored at rest in DRAM as FP8 E3M4, pre-scaled by 2^8 on
    the host (the 2^-8 descale is folded into w1, exactly).  This cuts
    the dominant HBM stream 4x vs f32: 32 MiB per core per pass.  The
    PE consumes fp8 at bf16 speed (no DoubleRow - E4M3 would lose too
    much precision), so the kernel is TensorE-bound at ~110 us/pass.
  - Fuse the two independent ARMA stacks: Y = [xbar_k0 @ w1_k0 |
    xbar_k1 @ w1_k1] is [N, 64], so fltr streams only once per
    iteration.
  - All big matmuls run transposed (out^T = Y^T @ fltr_m^T) so fltr is
    the 512-wide moving operand (128 elem/cycle); Y tiles are the
    stationary operand (weight loads hide under the previous matmul).
  - Iteration 0 needs no communication (x is replicated).  Pass 1 runs
    in two output-row halves (full-width fltr^T streams, 1 KiB DMA
    lines - narrower strips choke the HWDGE descriptor ring); each half
    feeds TWO 512-row PSUM accumulators and fires TWO small (64 KiB)
    Y1 all-gathers, so pass 2 can consume gather chunks as they land.
    A dummy warm-up collective at t=0 absorbs the one-time rendezvous
    barrier + Mesh warm-up (~60us) that would otherwise delay gather 0.
  - Pass 2 consumes the gathered chunks contraction-major (chunk 0..2
    feed all four output accumulators, chunk 3 is processed per output
    half so the first half's epilogue hides under the second half's
    stream); chunk 3 is not needed until ~85us after pass 2 starts,
    tolerating inter-core start skew.
  - Big fltr DMAs ride the sync-engine HWDGE ring; all small/latency
    DMAs ride the scalar-engine ring so they never queue behind a
    1 MiB fltr read; collectives keep the gpsimd queue.
  - relu positive homogeneity folds the final stack-mean 0.5 scale into
    the pass-2 activation; the host only shards/quantizes inputs and
    concatenates/transposes the [32, 2048] per-core outputs.
"""

import numpy as np
import ml_dtypes

import concourse.mybir as mybir
import concourse.tile as tile
from concourse import bacc
from concourse.bass_utils import run_bass_kernel_spmd

N = 16384            # nodes
F = 64               # input features
C = 32               # channels per stack
C2 = 2 * C           # fused channels (2 stacks)
NCORES = 8
R = N // NCORES      # fltr rows per core (2048)
P = 128              # partitions
NKT = N // P         # K tiles per full pass (128)
HW_ = R // 2         # 1024 output rows per half-array
CW = 512             # output rows per pass-1 chunk / PSUM accumulator
NCH = R // CW        # 4 pass-1 chunks (each with its own all-gather)
XCHUNK = 4096        # xT DMA chunk width
KB1 = 8              # K tiles per pass-1 fltr DMA (1 MiB fp8 reads)
FSCALE = 256.0       # power-of-2 fp8 pre-scale (folded into w1)

F32 = mybir.dt.float32
F32R = mybir.dt.float32r
BF16 = mybir.dt.bfloat16
F8 = mybir.dt.float8e3

_CACHE = {}


def _build():
    nc = bacc.Bacc(
        trn_type="TRN2", target_bir_lowering=False, debug=False,
        num_devices=NCORES,
    )
    fltrT0_e = nc.dram_tensor("fltrt0", [N, HW_], F8, kind="ExternalInput")
    fltrT1_e = nc.dram_tensor("fltrt1", [N, HW_], F8, kind="ExternalInput")
    xT_e = nc.dram_tensor("xt", [F, N], BF16, kind="ExternalInput")
    xtm_e = nc.dram_tensor("xtm", [F, R], F32, kind="ExternalInput")
    w1i0_e = nc.dram_tensor("w1i0", [F, C2], BF16, kind="ExternalInput")
    w1i1_e = nc.dram_tensor("w1i1", [C2, C2], BF16, kind="ExternalInput")
    w2i0_e = nc.dram_tensor("w2i0", [F, C2], F32, kind="ExternalInput")
    w2i1_e = nc.dram_tensor("w2i1", [F, C2], F32, kind="ExternalInput")
    bi0_e = nc.dram_tensor("bi0", [C2, 1], F32, kind="ExternalInput")
    bi1h_e = nc.dram_tensor("bi1h", [C2, 1], F32, kind="ExternalInput")
    out_e = nc.dram_tensor("out", [C, R], F32, kind="ExternalOutput")

    RG = [list(range(NCORES))]
    fltr_halves = [fltrT0_e, fltrT1_e]

    with tile.TileContext(nc) as tc:
        with (
            tc.tile_pool(name="wpool", bufs=1) as wpool,
            tc.tile_pool(name="xcpool", bufs=3) as xcpool,
            tc.tile_pool(name="y0pool", bufs=1) as y0pool,
            tc.tile_pool(name="ygpool", bufs=1) as ygpool,
            tc.tile_pool(name="fpool", bufs=8) as fpool,
            tc.tile_pool(name="xbpool", bufs=2) as xbpool,
            tc.tile_pool(name="ylpool", bufs=2) as ylpool,
            tc.tile_pool(name="opool", bufs=1) as opool,
            tc.tile_pool(name="pacc", bufs=4, space="PSUM") as pacc,
            tc.tile_pool(name="psmall", bufs=2, space="PSUM") as psmall,
            tc.tile_pool(name="dram", bufs=8, space="DRAM") as dram,
        ):
            # w1i0 and the first xT chunks first: they gate the first
            # Y0 matmul and thus the whole pass-1 PE start
            w1i0 = wpool.tile([F, C2], BF16)
            nc.scalar.dma_start(w1i0[:], w1i0_e[:])

            # dummy warm-up collective: absorbs the one-time rendezvous
            # barrier + collective warm-up under Y0/pass-1 compute so the
            # first real gather's data phase starts promptly (collectives
            # cannot read IO tensors, so bounce w1i0 through a DRAM tile)
            gwin = dram.tile([F, C2], BF16, name="gwin", tag="gwin")
            nc.scalar.dma_start(gwin[:], w1i0[:])
            gwout = dram.tile([NCORES * F, C2], BF16, name="gwout",
                              tag="gwout", addr_space="Shared")
            nc.gpsimd.collective_compute(
                "AllGather", mybir.AluOpType.bypass,
                replica_groups=RG,
                ins=[gwin[:].opt()], outs=[gwout[:].opt()],
            )
            xcs = []
            for g in range(2):
                xc = xcpool.tile([F, XCHUNK], BF16, name="xc")
                nc.scalar.dma_start(xc[:],
                                    xT_e[:, g * XCHUNK:(g + 1) * XCHUNK])
                xcs.append(xc)

            # remaining resident small tensors
            w1i1 = wpool.tile([C2, C2], BF16)  # block-diag [w1_k0i1, w1_k1i1]
            nc.scalar.dma_start(w1i1[:], w1i1_e[:])
            w2i0 = wpool.tile([F, C2], F32R)
            nc.scalar.dma_start(w2i0[:], w2i0_e[:].bitcast(F32R))
            bi0 = wpool.tile([C2, 1], F32)
            nc.scalar.dma_start(bi0[:], bi0_e[:])
            xm = wpool.tile([F, R], F32R)
            nc.scalar.dma_start(xm[:], xtm_e[:].bitcast(F32R))
            w2i1 = wpool.tile([F, C2], F32R)
            nc.scalar.dma_start(w2i1[:], w2i1_e[:].bitcast(F32R))
            bi1h = wpool.tile([C2, 1], F32)
            nc.scalar.dma_start(bi1h[:], bi1h_e[:])

            y0 = y0pool.tile([P, NKT, C2], BF16, tag="y0")  # node-major Y0

            # ---- Y0 = x @ [w1_k0i0 | w1_k1i0], node-major, cast to bf16 ----
            for g in range(N // XCHUNK):  # 4 groups of 32 kt
                if g < 2:
                    xc = xcs[g]
                else:
                    xc = xcpool.tile([F, XCHUNK], BF16, name="xc")
                    nc.scalar.dma_start(xc[:],
                                        xT_e[:, g * XCHUNK:(g + 1) * XCHUNK])
                for gg in range(XCHUNK // 1024):
                    ps0 = psmall.tile([P, 8, C2], F32, name="ps0", tag="ps0")
                    for i in range(8):
                        nc.tensor.matmul(
                            ps0[:, i, :],
                            xc[:, gg * 1024 + i * P:gg * 1024 + (i + 1) * P],
                            w1i0[:],
                            start=True, stop=True,
                        )
                    nc.vector.tensor_copy(
                        y0[:, (g * 4 + gg) * 8:(g * 4 + gg + 1) * 8, :],
                        ps0[:])

            # pass-2 gather-chunk tiles: yg tile c holds gather chunk c; row
            # b*128+p of gout_c is node (b//4)*2048 + c*512 + (b%4)*128 + p.
            yg = [ygpool.tile([P, NCORES * 4, C2], BF16, name=f"yg{c}",
                              tag=f"yg{c}") for c in range(NCH)]
            yg_issued = [False] * NCH
            gouts = []

            def issue_yg(c):
                # deferred issue: by the time it is queued the gather is
                # (normally) complete, so the scalar ring never head-blocks
                if not yg_issued[c]:
                    nc.scalar.dma_start(
                        yg[c][:],
                        gouts[c][:].rearrange("(b p) ch -> p b ch", p=P),
                    )
                    yg_issued[c] = True

            def issue_yg0():
                issue_yg(0)

            # ---- pass 1: two output-row halves (full-width 1 KiB DMA
            # ---- lines); each half fills two 512-row accumulators and
            # ---- fires two small all-gathers back-to-back
            for half in range(2):
                p1 = []
                for rc2 in range(2):
                    ck = half * 2 + rc2
                    acc = pacc.tile([C2, CW], F32, name=f"p1_{ck}",
                                    tag="acc")
                    nc.tensor.matmul(
                        acc[:],
                        w2i0[:],
                        xm[:, ck * CW:(ck + 1) * CW],
                        start=True, stop=False,
                    )
                    p1.append(acc)
                for ktb in range(NKT // KB1):
                    ft = fpool.tile([P, KB1, HW_], F8, name="ft", tag="ft")
                    nc.sync.dma_start(
                        ft[:],
                        fltr_halves[half][ktb * KB1 * P:(ktb + 1) * KB1 * P,
                                          :]
                        .rearrange("(b p) c -> p b c", p=P),
                    )
                    for b in range(KB1):
                        kt = ktb * KB1 + b
                        for rc2 in range(2):
                            nc.tensor.matmul(
                                p1[rc2][:],
                                y0[:, kt, :],
                                ft[:, b, rc2 * CW:(rc2 + 1) * CW],
                                start=False, stop=(kt == NKT - 1),
                            )

                # epilogue: relu -> Y1 chunks (bf16) -> two small all-gathers
                for rc2 in range(2):
                    y1q = ylpool.tile([P, 4, C2], BF16, name="y1q")
                    xb1 = xbpool.tile([C2, CW], BF16, name="xb1")
                    nc.scalar.activation(
                        xb1[:], p1[rc2][:],
                        mybir.ActivationFunctionType.Relu,
                        bias=bi0[:], scale=1.0,
                    )
                    for t in range(4):  # node-subtiles of 128 in the chunk
                        psy = psmall.tile([P, C2], F32, name="psy",
                                          tag="psy")
                        nc.tensor.matmul(
                            psy[:],
                            xb1[:, t * P:(t + 1) * P],
                            w1i1[:],
                            start=True, stop=True,
                        )
                        nc.vector.tensor_copy(y1q[:, t, :], psy[:])
                    gin = dram.tile([CW, C2], BF16, name="gin", tag="gin",
                                    bufs=4)
                    nc.scalar.dma_start(
                        gin[:].rearrange("(t p) ch -> p t ch", p=P),
                        y1q[:],
                    )
                    gout = dram.tile(
                        [NCORES * CW, C2], BF16, name="gout", tag="gout",
                        addr_space="Shared", bufs=4,
                    )
                    nc.gpsimd.collective_compute(
                        "AllGather", mybir.AluOpType.bypass,
                        replica_groups=RG,
                        ins=[gin[:].opt()], outs=[gout[:].opt()],
                    )
                    gouts.append(gout)
            # gather-0 finished well before pass-1 ends; load its yg tile
            # now so pass-2's first matmuls start without a long wait
            issue_yg(0)

            outT = opool.tile([C, R], F32)

            # ---- pass 2: contraction is gathered Y1, consumed chunk-major
            p2 = []
            for rc in range(NCH):
                acc = pacc.tile([C2, CW], F32, name=f"p2_{rc}", tag="acc")
                nc.tensor.matmul(
                    acc[:],
                    w2i1[:],
                    xm[:, rc * CW:(rc + 1) * CW],
                    start=True, stop=False,
                )
                p2.append(acc)

            def p2_block(c, j, hx, rcs, stop):
                # contraction rows [j*2048 + c*512, +512) of half-array hx
                ft = fpool.tile([P, 4, HW_], F8, name="ft2", tag="ft")
                nc.sync.dma_start(
                    ft[:],
                    fltr_halves[hx][j * R + c * CW:j * R + (c + 1) * CW, :]
                    .rearrange("(b p) c -> p b c", p=P),
                )
                for b in range(4):
                    for rc in rcs:
                        nc.tensor.matmul(
                            p2[rc][:],
                            yg[c][:, j * 4 + b, :],
                            ft[:, b, (rc % 2) * CW:(rc % 2 + 1) * CW],
                            start=False,
                            stop=stop and b == 3 and rc == rcs[-1],
                        )

            # phase A: chunks 0..2 feed all four output accumulators
            for c in range(3):
                if c > 0:
                    issue_yg(c)
                for j in range(NCORES):
                    p2_block(c, j, 0, [0, 1], False)
                    p2_block(c, j, 1, [2, 3], False)

            def p2_epilogue(rc):
                xb2 = xbpool.tile([C2, CW], F32, name="xb2")
                nc.scalar.activation(
                    xb2[:], p2[rc][:], mybir.ActivationFunctionType.Relu,
                    bias=bi1h[:], scale=0.5,
                )
                # partition-shift stack-1 half to base 0 (DMA), then add
                xs = xbpool.tile([C, CW], F32, name="xs")
                nc.scalar.dma_start(xs[:], xb2[C:C2, :])
                nc.vector.tensor_add(
                    outT[:, rc * CW:(rc + 1) * CW],
                    xb2[0:C, :], xs[:],
                )

            # phase B: chunk 3 per output half; the first half's epilogue
            # hides under the second half's matmul stream
            issue_yg(3)
            for j in range(NCORES):
                p2_block(3, j, 0, [0, 1], j == NCORES - 1)
            for rc in (0, 1):
                p2_epilogue(rc)
            nc.scalar.dma_start(out_e[:, 0:HW_], outT[:, 0:HW_])
            for j in range(NCORES):
                p2_block(3, j, 1, [2, 3], j == NCORES - 1)
            for rc in (2, 3):
                p2_epilogue(rc)
            nc.scalar.dma_start(out_e[:, HW_:R], outT[:, HW_:R])

    nc.compile()
    return nc


def kernel(**inputs):
    x = np.ascontiguousarray(np.asarray(inputs["x"], dtype=np.float32))
    fltr = np.ascontiguousarray(np.asarray(inputs["fltr"], dtype=np.float32))

    def cat(a, b, axis=1):
        return np.ascontiguousarray(
            np.concatenate(
                [np.asarray(a, np.float32), np.asarray(b, np.float32)],
                axis=axis,
            )
        )

    f8 = ml_dtypes.float8_e3m4
    bf = ml_dtypes.bfloat16
    w1i0 = np.ascontiguousarray(
        (cat(inputs["k0i0_w1"], inputs["k1i0_w1"]) / FSCALE).astype(bf))
    w1i1f = np.zeros((C2, C2), dtype=np.float32)
    w1i1f[0:C, 0:C] = np.asarray(inputs["k0i1_w1"], np.float32)
    w1i1f[C:C2, C:C2] = np.asarray(inputs["k1i1_w1"], np.float32)
    w1i1 = np.ascontiguousarray((w1i1f / FSCALE).astype(bf))
    w2i0 = cat(inputs["k0i0_w2"], inputs["k1i0_w2"])
    w2i1 = cat(inputs["k0i1_w2"], inputs["k1i1_w2"])
    bi0 = cat(inputs["k0i0_b"], inputs["k1i0_b"], axis=0)[:, None]
    bi1h = 0.5 * cat(inputs["k0i1_b"], inputs["k1i1_b"], axis=0)[:, None]
    bi1h = np.ascontiguousarray(bi1h)
    xT = np.ascontiguousarray(x.T.astype(bf))
    # fp8 E3M4 fltr at rest: transpose per core, scale by 2^8 (descale is
    # folded into w1i0/w1i1 above; values land in [-10.9, 10.9] < 15.5 max)
    fltrs = (fltr * np.float32(FSCALE)).astype(f8)

    if "nc" not in _CACHE:
        _CACHE["nc"] = _build()
    nc = _CACHE["nc"]

    in_maps = []
    for m in range(NCORES):
        rows = slice(m * R, (m + 1) * R)
        in_maps.append({
            "fltrt0": np.ascontiguousarray(fltrs[m * R:m * R + HW_, :].T),
            "fltrt1": np.ascontiguousarray(fltrs[m * R + HW_:(m + 1) * R, :].T),
            "xt": xT,
            "xtm": np.ascontiguousarray(x[rows, :].T),
            "w1i0": w1i0, "w1i1": w1i1, "w2i0": w2i0, "w2i1": w2i1,
            "bi0": bi0, "bi1h": bi1h,
        })

    import os
    import time
    trace = os.environ.get("ARMA_TRACE") == "1"
    last_exc = None
    for attempt in range(3):
        try:
            res = run_bass_kernel_spmd(
                nc, in_maps, core_ids=list(range(NCORES)), trace=trace,
            )
            break
        except Exception as e:  # transient NRT device errors: retry
            last_exc = e
            time.sleep(5.0)
    else:
        raise last_exc
    _CACHE["last_results"] = res
    out = np.concatenate(
        [np.asarray(res.results[m]["out"]).T for m in range(NCORES)], axis=0
    )
    return out


# revision 27
# speedup vs baseline: 2.6474x; 1.0012x over previous
"""Distributed ARMAConv kernel for 8 TRN2 NeuronCores (Bass/Tile).

Reference computation (N=16384 nodes, F=64 in-feats, C=32 channels,
K=2 stacks, T=2 iterations):
    for each stack k:  xbar = x
        for i in 0..1: xbar = relu(fltr @ (xbar @ w1) + x @ w2 + b)
    out = mean over stacks                                  -> [N, 32]

Strategy:
  - Row-shard fltr across 8 cores; core m holds fltr[rows_m, :] stored
    TRANSPOSED (contraction-major, two contiguous half-arrays) so every
    TensorE tile is a large contiguous DMA read.
  - fltr is stored at rest in DRAM as FP8 E3M4, pre-scaled by 2^8 on
    the host (the 2^-8 descale is folded into w1, exactly).  This cuts
    the dominant HBM stream 4x vs f32: 32 MiB per core per pass.  The
    PE consumes fp8 at bf16 speed (no DoubleRow - E4M3 would lose too
    much precision), so the kernel is TensorE-bound at ~110 us/pass.
  - Fuse the two independent ARMA stacks: Y = [xbar_k0 @ w1_k0 |
    xbar_k1 @ w1_k1] is [N, 64], so fltr streams only once per
    iteration.
  - All big matmuls run transposed (out^T = Y^T @ fltr_m^T) so fltr is
    the 512-wide moving operand (128 elem/cycle); Y tiles are the
    stationary operand (weight loads hide under the previous matmul).
  - Iteration 0 needs no communication (x is replicated).  Pass 1 runs
    in two output-row halves (full-width fltr^T streams, 1 KiB DMA
    lines - narrower strips choke the HWDGE descriptor ring); each half
    feeds TWO 512-row PSUM accumulators and fires TWO small (64 KiB)
    Y1 all-gathers, so pass 2 can consume gather chunks as they land.
    A dummy warm-up collective at t=0 absorbs the one-time rendezvous
    barrier + Mesh warm-up (~60us) that would otherwise delay gather 0.
  - Pass 2 consumes the gathered chunks contraction-major (chunk 0..2
    feed all four output accumulators, chunk 3 is processed per output
    half so the first half's epilogue hides under the second half's
    stream); chunk 3 is not needed until ~85us after pass 2 starts,
    tolerating inter-core start skew.
  - Big fltr DMAs ride the sync-engine HWDGE ring; all small/latency
    DMAs ride the scalar-engine ring so they never queue behind a
    1 MiB fltr read; collectives keep the gpsimd queue.
  - relu positive homogeneity folds the final stack-mean 0.5 scale into
    the pass-2 activation; the host only shards/quantizes inputs and
    concatenates/transposes the [32, 2048] per-core outputs.
"""

import numpy as np
import ml_dtypes

import concourse.mybir as mybir
import concourse.tile as tile
from concourse import bacc
from concourse.bass_utils import run_bass_kernel_spmd

N = 16384            # nodes
F = 64               # input features
C = 32               # channels per stack
C2 = 2 * C           # fused channels (2 stacks)
NCORES = 8
R = N // NCORES      # fltr rows per core (2048)
P = 128              # partitions
NKT = N // P         # K tiles per full pass (128)
HW_ = R // 2         # 1024 output rows per half-array
CW = 512             # output rows per pass-1 chunk / PSUM accumulator
NCH = R // CW        # 4 pass-1 chunks (each with its own all-gather)
KB1 = 4              # K tiles per pass-1 fltr DMA (512 KiB fp8 reads;
                     # 512-row tiles match pass-2's contraction blocks)
FSCALE = 256.0       # power-of-2 fp8 pre-scale (folded into w1)

F32 = mybir.dt.float32
F32R = mybir.dt.float32r
BF16 = mybir.dt.bfloat16
F8 = mybir.dt.float8e3

_CACHE = {}


def _build():
    nc = bacc.Bacc(
        trn_type="TRN2", target_bir_lowering=False, debug=False,
        num_devices=NCORES,
    )
    fltrT0_e = nc.dram_tensor("fltrt0", [N, HW_], F8, kind="ExternalInput")
    fltrT1_e = nc.dram_tensor("fltrt1", [N, HW_], F8, kind="ExternalInput")
    xT_e = nc.dram_tensor("xt", [F, N], BF16, kind="ExternalInput")
    xtm_e = nc.dram_tensor("xtm", [F, R], F32, kind="ExternalInput")
    w1i0_e = nc.dram_tensor("w1i0", [F, C2], BF16, kind="ExternalInput")
    w1i1_e = nc.dram_tensor("w1i1", [C2, C2], BF16, kind="ExternalInput")
    w2i0_e = nc.dram_tensor("w2i0", [F, C2], F32, kind="ExternalInput")
    w2i1_e = nc.dram_tensor("w2i1", [F, C2], F32, kind="ExternalInput")
    bi0_e = nc.dram_tensor("bi0", [C2, 1], F32, kind="ExternalInput")
    bi1h_e = nc.dram_tensor("bi1h", [C2, 1], F32, kind="ExternalInput")
    out_e = nc.dram_tensor("out", [C, R], F32, kind="ExternalOutput")

    RG = [list(range(NCORES))]
    fltr_halves = [fltrT0_e, fltrT1_e]

    with tile.TileContext(nc) as tc:
        with (
            tc.tile_pool(name="wpool", bufs=1) as wpool,
            tc.tile_pool(name="kpool", bufs=1) as kpool,
            tc.tile_pool(name="y0pool", bufs=1) as y0pool,
            tc.tile_pool(name="ygpool", bufs=1) as ygpool,
            tc.tile_pool(name="fpool", bufs=8) as fpool,
            tc.tile_pool(name="xbpool", bufs=2) as xbpool,
            tc.tile_pool(name="ylpool", bufs=2) as ylpool,
            tc.tile_pool(name="opool", bufs=1) as opool,
            tc.tile_pool(name="pacc", bufs=4, space="PSUM") as pacc,
            tc.tile_pool(name="psmall", bufs=2, space="PSUM") as psmall,
            tc.tile_pool(name="dram", bufs=8, space="DRAM") as dram,
        ):
            # w1i0 and the full xT first: they gate the first Y0 matmul and
            # thus the whole pass-1 PE start.  xT rides the (otherwise
            # still-empty) sync ring ahead of the fltr stream so Y0 is
            # never DMA-gated.
            w1i0 = wpool.tile([F, C2], BF16)
            nc.scalar.dma_start(w1i0[:], w1i0_e[:])
            xf = wpool.tile([F, N], BF16)
            for g in range(2):
                nc.sync.dma_start(xf[:, g * (N // 2):(g + 1) * (N // 2)],
                                  xT_e[:, g * (N // 2):(g + 1) * (N // 2)])

            # dummy warm-up collective: absorbs the one-time rendezvous
            # barrier + collective warm-up under Y0/pass-1 compute so the
            # first real gather's data phase starts promptly (collectives
            # cannot read IO tensors, so bounce w1i0 through a DRAM tile)
            gwin = dram.tile([F, C2], BF16, name="gwin", tag="gwin")
            nc.scalar.dma_start(gwin[:], w1i0[:])
            gwout = dram.tile([NCORES * F, C2], BF16, name="gwout",
                              tag="gwout", addr_space="Shared")
            nc.gpsimd.collective_compute(
                "AllGather", mybir.AluOpType.bypass,
                replica_groups=RG,
                ins=[gwin[:].opt()], outs=[gwout[:].opt()],
            )

            # remaining resident small tensors
            w1i1 = wpool.tile([C2, C2], BF16)  # block-diag [w1_k0i1, w1_k1i1]
            nc.scalar.dma_start(w1i1[:], w1i1_e[:])
            w2i0 = wpool.tile([F, C2], F32R)
            nc.scalar.dma_start(w2i0[:], w2i0_e[:].bitcast(F32R))
            bi0 = wpool.tile([C2, 1], F32)
            nc.scalar.dma_start(bi0[:], bi0_e[:])
            xm = wpool.tile([F, R], F32R)
            nc.scalar.dma_start(xm[:], xtm_e[:].bitcast(F32R))
            w2i1 = wpool.tile([F, C2], F32R)
            nc.scalar.dma_start(w2i1[:], w2i1_e[:].bitcast(F32R))
            bi1h = wpool.tile([C2, 1], F32)
            nc.scalar.dma_start(bi1h[:], bi1h_e[:])

            y0 = y0pool.tile([P, NKT, C2], BF16, tag="y0")  # node-major Y0

            # ---- Y0 = x @ [w1_k0i0 | w1_k1i0], node-major, cast to bf16 ----
            for g in range(16):  # 16 groups of 8 kt
                ps0 = psmall.tile([P, 8, C2], F32, name="ps0", tag="ps0")
                for i in range(8):
                    nc.tensor.matmul(
                        ps0[:, i, :],
                        xf[:, g * 1024 + i * P:g * 1024 + (i + 1) * P],
                        w1i0[:],
                        start=True, stop=True,
                    )
                nc.vector.tensor_copy(y0[:, g * 8:(g + 1) * 8, :], ps0[:])

            # pass-2 gather-chunk tiles: yg tile c holds gather chunk c; row
            # b*128+p of gout_c is node (b//4)*2048 + c*512 + (b%4)*128 + p.
            yg = [ygpool.tile([P, NCORES * 4, C2], BF16, name=f"yg{c}",
                              tag=f"yg{c}") for c in range(NCH)]
            yg_issued = [False] * NCH
            gouts = []

            def issue_yg(c):
                # deferred issue: by the time it is queued the gather is
                # (normally) complete, so the scalar ring never head-blocks
                if not yg_issued[c]:
                    nc.scalar.dma_start(
                        yg[c][:],
                        gouts[c][:].rearrange("(b p) ch -> p b ch", p=P),
                    )
                    yg_issued[c] = True

            def issue_yg0():
                issue_yg(0)

            # ---- pass 1: two output-row halves (full-width 1 KiB DMA
            # ---- lines); each half fills two 512-row accumulators and
            # ---- fires two small all-gathers back-to-back
            kept = {}
            for half in range(2):
                p1 = []
                for rc2 in range(2):
                    ck = half * 2 + rc2
                    acc = pacc.tile([C2, CW], F32, name=f"p1_{ck}",
                                    tag="acc")
                    nc.tensor.matmul(
                        acc[:],
                        w2i0[:],
                        xm[:, ck * CW:(ck + 1) * CW],
                        start=True, stop=False,
                    )
                    p1.append(acc)
                for ktb in range(NKT // KB1):
                    if ktb % 4 == 3:
                        # contraction rows [j*2048+1536, +512): pass-2's
                        # phase-B (chunk 3) blocks - pin them in SBUF so
                        # phase B needs no fltr DMA at all
                        ft = kpool.tile([P, KB1, HW_], F8, name="ftk",
                                        tag="ftk", bufs=16)
                        kept[(half, ktb // 4)] = ft
                    else:
                        ft = fpool.tile([P, KB1, HW_], F8, name="ft",
                                        tag="ft")
                    nc.sync.dma_start(
                        ft[:],
                        fltr_halves[half][ktb * KB1 * P:(ktb + 1) * KB1 * P,
                                          :]
                        .rearrange("(b p) c -> p b c", p=P),
                    )
                    for b in range(KB1):
                        kt = ktb * KB1 + b
                        for rc2 in range(2):
                            nc.tensor.matmul(
                                p1[rc2][:],
                                y0[:, kt, :],
                                ft[:, b, rc2 * CW:(rc2 + 1) * CW],
                                start=False, stop=(kt == NKT - 1),
                            )

                # epilogue: relu -> Y1 chunks (bf16) -> two small all-gathers
                for rc2 in range(2):
                    y1q = ylpool.tile([P, 4, C2], BF16, name="y1q")
                    xb1 = xbpool.tile([C2, CW], BF16, name="xb1")
                    nc.scalar.activation(
                        xb1[:], p1[rc2][:],
                        mybir.ActivationFunctionType.Relu,
                        bias=bi0[:], scale=1.0,
                    )
                    for t in range(4):  # node-subtiles of 128 in the chunk
                        psy = psmall.tile([P, C2], F32, name="psy",
                                          tag="psy")
                        nc.tensor.matmul(
                            psy[:],
                            xb1[:, t * P:(t + 1) * P],
                            w1i1[:],
                            start=True, stop=True,
                        )
                        nc.vector.tensor_copy(y1q[:, t, :], psy[:])
                    gin = dram.tile([CW, C2], BF16, name="gin", tag="gin",
                                    bufs=4)
                    nc.scalar.dma_start(
                        gin[:].rearrange("(t p) ch -> p t ch", p=P),
                        y1q[:],
                    )
                    gout = dram.tile(
                        [NCORES * CW, C2], BF16, name="gout", tag="gout",
                        addr_space="Shared", bufs=4,
                    )
                    nc.gpsimd.collective_compute(
                        "AllGather", mybir.AluOpType.bypass,
                        replica_groups=RG,
                        ins=[gin[:].opt()], outs=[gout[:].opt()],
                    )
                    gouts.append(gout)
            # gather-0 finished well before pass-1 ends; load its yg tile
            # now so pass-2's first matmuls start without a long wait
            issue_yg(0)

            outT = opool.tile([C, R], F32)

            # ---- pass 2: contraction is gathered Y1, consumed chunk-major
            p2 = []
            for rc in range(NCH):
                acc = pacc.tile([C2, CW], F32, name=f"p2_{rc}", tag="acc")
                nc.tensor.matmul(
                    acc[:],
                    w2i1[:],
                    xm[:, rc * CW:(rc + 1) * CW],
                    start=True, stop=False,
                )
                p2.append(acc)

            def p2_block(c, j, hx, rcs, stop):
                # contraction rows [j*2048 + c*512, +512) of half-array hx
                if c == 3:
                    ft = kept[(hx, j)]  # pinned in SBUF since pass 1
                else:
                    ft = fpool.tile([P, 4, HW_], F8, name="ft2", tag="ft")
                    nc.sync.dma_start(
                        ft[:],
                        fltr_halves[hx][j * R + c * CW:j * R + (c + 1) * CW,
                                        :]
                        .rearrange("(b p) c -> p b c", p=P),
                    )
                for b in range(4):
                    for rc in rcs:
                        nc.tensor.matmul(
                            p2[rc][:],
                            yg[c][:, j * 4 + b, :],
                            ft[:, b, (rc % 2) * CW:(rc % 2 + 1) * CW],
                            start=False,
                            stop=stop and b == 3 and rc == rcs[-1],
                        )

            # phase A: chunks 0..2 feed all four output accumulators
            for c in range(3):
                if c > 0:
                    issue_yg(c)
                for j in range(NCORES):
                    p2_block(c, j, 0, [0, 1], False)
                    p2_block(c, j, 1, [2, 3], False)

            def p2_epilogue(rc):
                xb2 = xbpool.tile([C2, CW], F32, name="xb2")
                nc.scalar.activation(
                    xb2[:], p2[rc][:], mybir.ActivationFunctionType.Relu,
                    bias=bi1h[:], scale=0.5,
                )
                # partition-shift stack-1 half to base 0 (DMA), then add
                xs = xbpool.tile([C, CW], F32, name="xs")
                nc.scalar.dma_start(xs[:], xb2[C:C2, :])
                nc.vector.tensor_add(
                    outT[:, rc * CW:(rc + 1) * CW],
                    xb2[0:C, :], xs[:],
                )

            # phase B: chunk 3 per output half; the first half's epilogue
            # hides under the second half's matmul stream
            issue_yg(3)
            for j in range(NCORES):
                p2_block(3, j, 0, [0, 1], j == NCORES - 1)
            for rc in (0, 1):
                p2_epilogue(rc)
            nc.scalar.dma_start(out_e[:, 0:HW_], outT[:, 0:HW_])
            for j in range(NCORES):
                p2_block(3, j, 1, [2, 3], j == NCORES - 1)
            for rc in (2, 3):
                p2_epilogue(rc)
            nc.scalar.dma_start(out_e[:, HW_:R], outT[:, HW_:R])

    nc.compile()
    return nc


def kernel(**inputs):
    x = np.ascontiguousarray(np.asarray(inputs["x"], dtype=np.float32))
    fltr = np.ascontiguousarray(np.asarray(inputs["fltr"], dtype=np.float32))

    def cat(a, b, axis=1):
        return np.ascontiguousarray(
            np.concatenate(
                [np.asarray(a, np.float32), np.asarray(b, np.float32)],
                axis=axis,
            )
        )

    f8 = ml_dtypes.float8_e3m4
    bf = ml_dtypes.bfloat16
    w1i0 = np.ascontiguousarray(
        (cat(inputs["k0i0_w1"], inputs["k1i0_w1"]) / FSCALE).astype(bf))
    w1i1f = np.zeros((C2, C2), dtype=np.float32)
    w1i1f[0:C, 0:C] = np.asarray(inputs["k0i1_w1"], np.float32)
    w1i1f[C:C2, C:C2] = np.asarray(inputs["k1i1_w1"], np.float32)
    w1i1 = np.ascontiguousarray((w1i1f / FSCALE).astype(bf))
    w2i0 = cat(inputs["k0i0_w2"], inputs["k1i0_w2"])
    w2i1 = cat(inputs["k0i1_w2"], inputs["k1i1_w2"])
    bi0 = cat(inputs["k0i0_b"], inputs["k1i0_b"], axis=0)[:, None]
    bi1h = 0.5 * cat(inputs["k0i1_b"], inputs["k1i1_b"], axis=0)[:, None]
    bi1h = np.ascontiguousarray(bi1h)
    xT = np.ascontiguousarray(x.T.astype(bf))
    # fp8 E3M4 fltr at rest: transpose per core, scale by 2^8 (descale is
    # folded into w1i0/w1i1 above; values land in [-10.9, 10.9] < 15.5 max)
    fltrs = (fltr * np.float32(FSCALE)).astype(f8)

    if "nc" not in _CACHE:
        _CACHE["nc"] = _build()
    nc = _CACHE["nc"]

    in_maps = []
    for m in range(NCORES):
        rows = slice(m * R, (m + 1) * R)
        in_maps.append({
            "fltrt0": np.ascontiguousarray(fltrs[m * R:m * R + HW_, :].T),
            "fltrt1": np.ascontiguousarray(fltrs[m * R + HW_:(m + 1) * R, :].T),
            "xt": xT,
            "xtm": np.ascontiguousarray(x[rows, :].T),
            "w1i0": w1i0, "w1i1": w1i1, "w2i0": w2i0, "w2i1": w2i1,
            "bi0": bi0, "bi1h": bi1h,
        })

    import os
    import time
    trace = os.environ.get("ARMA_TRACE") == "1"
    last_exc = None
    for attempt in range(3):
        try:
            res = run_bass_kernel_spmd(
                nc, in_maps, core_ids=list(range(NCORES)), trace=trace,
            )
            break
        except Exception as e:  # transient NRT device errors: retry
            last_exc = e
            time.sleep(5.0)
    else:
        raise last_exc
    _CACHE["last_results"] = res
    out = np.concatenate(
        [np.asarray(res.results[m]["out"]).T for m in range(NCORES)], axis=0
    )
    return out
